# revision 24
# baseline (speedup 1.0000x reference)
"""Trainium2 Bass kernel for nn_AttentionBlock (4x256x64x64 self-attention).

Sharding: 8 cores = 4 batches x 2 query-halves. Each core computes, for its
batch b and query half h:
  k   = fold_bn(Wk) @ x[b] + bk'            [64, 4096]   (keys, full batch)
  q   = fold_bn(Wk) @ xq + bk'              [64, 2048]   (queries, this half)
  vT  = x[b].T @ Wv.T                       [4096, 256]  (j on partitions)
  E'  = k.T q - M_i  (augmented contraction row carries -M_i)
  P   = exp(E')  (stable: M_i = 0.5*(||q_i||^2 + max_j ||k_j||^2) >= max_j E_ij)
  out = (P.T @ [vT | 1]) -> unnormalized out + denominator column, then
        out[i, :] /= denom_i
Softmax normalization is shift-invariant, so using the norm bound M_i instead
of the exact row max gives the identical result while never overflowing.

The kernel emits E' transposed ([j, i] layout) so the exp output feeds the
P.T @ vT matmul directly with no on-chip transposes. bv is added host-side
(it is a per-channel additive constant on the output); BN is folded into Wk.
"""

import numpy as np

import concourse.bass as bass
import concourse.bacc as bacc
import concourse.tile as tile
import concourse.mybir as mybir
from concourse.bass_utils import run_bass_kernel_spmd

B, C, HH, WW = 4, 256, 64, 64
HW = HH * WW          # 4096
CK, CV = 64, 256
P = 128
QH = HW // 2          # 2048 queries per core
NCORES = 8
BN_EPS = 1e-5

NJ = HW // P          # 32 key chunks
IBS = 512             # i-block size (E-matmul moving dim)
NIB = QH // IBS       # 4 i-blocks
NQ = IBS // P         # 4 i-chunks (128) per i-block
KC = 512              # hw chunk for the k/q projection matmuls
LAG = 2               # software pipeline depth (E/exp ahead of PV)

F32 = mybir.dt.float32
F32R = mybir.dt.float32r
BF16 = mybir.dt.bfloat16
EXP = mybir.ActivationFunctionType.Exp
AX = mybir.AxisListType.X
ADD = mybir.AluOpType.add
MUL = mybir.AluOpType.mult
DIV = mybir.AluOpType.divide


def _emit(tc, xb, xq, wkT, bk, wvT, out):
    from contextlib import ExitStack

    nc = tc.nc
    with ExitStack() as ctx:
        consts = ctx.enter_context(tc.tile_pool(name="consts", bufs=1))
        big = ctx.enter_context(tc.tile_pool(name="big", bufs=1))
        work = ctx.enter_context(tc.tile_pool(name="work", bufs=4))

        # ---- constants -------------------------------------------------
        wk_sb = consts.tile([P, 2, CK], BF16)
        nc.sync.dma_start(wk_sb, wkT.rearrange("(o p) c -> p o c", p=P))
        wv_sb = consts.tile([P, 2, CV], BF16)
        bk_sb = consts.tile([CK, 1], F32)
        ones_f32 = consts.tile([P, 64], F32)
        nc.vector.memset(ones_f32, 1.0)
        ones_row = consts.tile([1, HW], F32)
        nc.gpsimd.memset(ones_row, 1.0)
        ones64 = consts.tile([CK, 2], BF16)
        nc.vector.tensor_copy(ones64, ones_f32[0:CK, 0:2])

        # ---- big persistent SBUF tensors -------------------------------
        xb_sb = big.tile([P, 2, HW], BF16)
        xq_sb = big.tile([P, 2, QH], BF16)
        kj = big.tile([CK + 1, HW], BF16)    # keys; row 64 = ones
        ki = big.tile([CK + 1, QH], BF16)    # queries; row 64 = -M_i
        sqk = big.tile([CK, HW], BF16)
        sqq = big.tile([CK, QH], BF16)
        n2q = big.tile([1, QH], F32)
        mx8 = big.tile([1, HW // KC], F32)   # per-chunk maxima of ||k_j||^2
        mx = consts.tile([1, 1], F32)
        vt = big.tile([P, NJ, CV + 2], BF16)  # vT tiles; cols 256,257 = ones

        nc.vector.tensor_copy(vt[:, :, CV:CV + 2],
                              ones_f32.rearrange("p (a b) -> p a b", b=2)[:, 0:NJ, :])

        xbr = xb.rearrange("(o p) f -> p o f", p=P)
        xqr = xq.rearrange("(o p) f -> p o f", p=P)

        # ---- DMA: keys first (they gate the longest chain) -------------
        NXB = 8
        bs = HW // NXB
        nc.sync.dma_start(xb_sb[:, :, 0:bs], xbr[:, :, 0:bs])
        nc.sync.dma_start(bk_sb, bk)
        nc.sync.dma_start(wv_sb, wvT.rearrange("(o p) c -> p o c", p=P))
        for t in range(1, NXB):
            nc.sync.dma_start(xb_sb[:, :, t * bs:(t + 1) * bs],
                              xbr[:, :, t * bs:(t + 1) * bs])
        NXQ = 4
        qs = QH // NXQ
        for t in range(NXQ):
            nc.sync.dma_start(xq_sb[:, :, t * qs:(t + 1) * qs],
                              xqr[:, :, t * qs:(t + 1) * qs])

        # ---- prologue: k/v/norm production (own psum pool, closed after)
        with tc.tile_pool(name="pool_pre", bufs=2, space="PSUM") as pool_pre, \
                tc.tile_pool(name="pool_v", bufs=2, space="PSUM") as pool_v:
            # key side: xb -> kj -> sqk -> max ||k||^2; also vT
            for t in range(HW // KC):
                sl = slice(t * KC, (t + 1) * KC)
                ps = pool_pre.tile([CK, KC], F32, tag="kps", name=f"kps_{t}")
                for o in range(2):
                    nc.tensor.matmul(ps, lhsT=wk_sb[:, o, :],
                                     rhs=xb_sb[:, o, sl],
                                     start=(o == 0), stop=(o == 1))
                nc.vector.tensor_scalar_add(kj[0:CK, sl], ps, bk_sb)
                nc.vector.tensor_mul(sqk[:, sl], kj[0:CK, sl], kj[0:CK, sl])
                ps2 = pool_pre.tile([2, KC], F32, tag="n2", name=f"n2kps_{t}")
                nc.tensor.matmul(ps2, lhsT=ones64,
                                 rhs=sqk[:, sl], start=True, stop=True)
                nc.vector.reduce_max(mx8[:, t:t + 1], ps2[0:1, :], axis=AX)
                for jc in range(t * 4, t * 4 + 4):
                    vps = pool_v.tile([P, CV], F32, tag="v", name=f"vps_{jc}")
                    for o in range(2):
                        nc.tensor.matmul(vps,
                                         lhsT=xb_sb[:, o, jc * P:(jc + 1) * P],
                                         rhs=wv_sb[:, o, :],
                                         start=(o == 0), stop=(o == 1))
                    if jc % 2 == 0:
                        nc.vector.tensor_copy(vt[:, jc, 0:CV], vps)
                    else:
                        nc.scalar.copy(vt[:, jc, 0:CV], vps)

            # augmentation row of the key side (feeds every E matmul lhsT)
            nc.gpsimd.tensor_copy(kj[CK:CK + 1, :], ones_row)

            # query side: xq -> ki -> sqq -> n2q
            for t in range(QH // KC):
                sl = slice(t * KC, (t + 1) * KC)
                ps = pool_pre.tile([CK, KC], F32, tag="kps", name=f"qps_{t}")
                for o in range(2):
                    nc.tensor.matmul(ps, lhsT=wk_sb[:, o, :],
                                     rhs=xq_sb[:, o, sl],
                                     start=(o == 0), stop=(o == 1))
                nc.vector.tensor_scalar_add(ki[0:CK, sl], ps, bk_sb)
                nc.vector.tensor_mul(sqq[:, sl], ki[0:CK, sl], ki[0:CK, sl])
                ps2 = pool_pre.tile([2, KC], F32, tag="n2", name=f"n2qps_{t}")
                nc.tensor.matmul(ps2, lhsT=ones64,
                                 rhs=sqq[:, sl], start=True, stop=True)
                nc.vector.tensor_copy(n2q[:, sl], ps2[0:1, :])

            nc.vector.reduce_max(mx, mx8, axis=AX)
            # -M_i = -0.5 * (n2q + mx), per i-block so E can start early
            for ib in range(NIB):
                sl = slice(ib * IBS, (ib + 1) * IBS)
                nc.vector.tensor_scalar(ki[CK:CK + 1, sl], n2q[:, sl],
                                        scalar1=mx, scalar2=-0.5, op0=ADD, op1=MUL)

        pool_e = ctx.enter_context(tc.tile_pool(name="pool_e", bufs=2, space="PSUM"))
        pool_o = ctx.enter_context(tc.tile_pool(name="pool_o", bufs=6, space="PSUM"))

        # ---- main attention loop (software-pipelined) ------------------
        pt_tiles = {}
        ops = {}

        def emit_e_exp(t):
            ib, jc = divmod(t, NJ)
            eps_t = pool_e.tile([P, IBS], F32, tag="e", name=f"eps_{ib}_{jc}")
            nc.tensor.matmul(eps_t,
                             lhsT=kj[:, jc * P:(jc + 1) * P],
                             rhs=ki[:, ib * IBS:(ib + 1) * IBS],
                             start=True, stop=True)
            pt = work.tile([P, IBS], BF16, tag="pt", name=f"pt_{ib}_{jc}")
            nc.scalar.activation(pt, eps_t, EXP)
            pt_tiles[t] = pt

        def emit_pv(t):
            ib, jc = divmod(t, NJ)
            if jc == 0:
                ops[ib] = [pool_o.tile([P, CV + 2], F32, tag="o",
                                       name=f"ops_{ib}_{q}") for q in range(NQ)]
            pt = pt_tiles.pop(t)
            for q in range(NQ):
                nc.tensor.matmul(ops[ib][q],
                                 lhsT=pt[:, q * P:(q + 1) * P],
                                 rhs=vt[:, jc, :],
                                 start=(jc == 0), stop=(jc == NJ - 1))
            if jc + 1 == NJ:
                for q in range(NQ):
                    ic = ib * NQ + q
                    r = work.tile([P, 1], F32, tag="r", name=f"r_{ib}_{q}")
                    nc.vector.reciprocal(r, ops[ib][q][:, CV:CV + 1])
                    ob = work.tile([P, CV], F32, tag="ob", name=f"ob_{ib}_{q}")
                    nc.vector.tensor_scalar_mul(ob, ops[ib][q][:, 0:CV], r)
                    nc.sync.dma_start(out[ic * P:(ic + 1) * P, :], ob)

        total = NIB * NJ
        for t in range(total):
            emit_e_exp(t)
            if t >= LAG:
                emit_pv(t - LAG)
        for t in range(total - LAG, total):
            emit_pv(t)


def build_nc(loop_n=None):
    nc = bacc.Bacc(trn_type="TRN2")
    xb_d = nc.dram_tensor("xb", [C, HW], BF16, kind="ExternalInput")
    xq_d = nc.dram_tensor("xq", [C, QH], BF16, kind="ExternalInput")
    wk_d = nc.dram_tensor("wkT", [C, CK], BF16, kind="ExternalInput")
    bk_d = nc.dram_tensor("bk", [CK, 1], F32, kind="ExternalInput")
    wv_d = nc.dram_tensor("wvT", [C, CV], BF16, kind="ExternalInput")
    out_d = nc.dram_tensor("out", [QH, CV], F32, kind="ExternalOutput")
    args = (xb_d[:], xq_d[:], wk_d[:], bk_d[:], wv_d[:], out_d[:])
    with tile.TileContext(nc) as tc:
        if loop_n is None:
            _emit(tc, *args)
        else:
            with tc.For_i(0, loop_n, 1,
                          hint_engines=(mybir.EngineType.PE,
                                        mybir.EngineType.Activation,
                                        mybir.EngineType.DVE)):
                _emit(tc, *args)
    nc.finalize()
    return nc


_NC = None


def get_nc():
    global _NC
    if _NC is None:
        _NC = build_nc()
    return _NC


def build_in_maps(inputs):
    x = np.ascontiguousarray(np.asarray(inputs["x"], np.float32))
    Wk = np.asarray(inputs["Wk"], np.float32)
    bk = np.asarray(inputs["bk"], np.float32)
    gamma = np.asarray(inputs["bn_gamma"], np.float32)
    beta = np.asarray(inputs["bn_beta"], np.float32)
    mean = np.asarray(inputs["bn_mean"], np.float32)
    var = np.asarray(inputs["bn_var"], np.float32)
    Wv = np.asarray(inputs["Wv"], np.float32)

    inv = gamma / np.sqrt(var + BN_EPS)
    wk_eff = (inv[:, None] * Wk).astype(np.float32)
    bk_eff = (inv * bk + (beta - mean * inv)).astype(np.float32)

    import ml_dtypes
    bf = ml_dtypes.bfloat16
    wkT = np.ascontiguousarray(wk_eff.T.astype(bf))     # [C, CK]
    wvT = np.ascontiguousarray(Wv.T.astype(bf))         # [C, CV]
    bk2 = np.ascontiguousarray(bk_eff.reshape(CK, 1))

    in_maps = []
    for core in range(NCORES):
        b, h = divmod(core, 2)
        xb = np.ascontiguousarray(x[b].reshape(C, HW).astype(bf))
        xq = np.ascontiguousarray(xb[:, h * QH:(h + 1) * QH])
        in_maps.append({"xb": xb, "xq": xq, "wkT": wkT, "bk": bk2, "wvT": wvT})
    return in_maps


def kernel(**inputs):
    bv = np.asarray(inputs["bv"], np.float32)
    in_maps = build_in_maps(inputs)
    nc = get_nc()
    res = run_bass_kernel_spmd(nc, in_maps, core_ids=list(range(NCORES)))
    out = np.empty((B, CV, HW), np.float32)
    for core in range(NCORES):
        b, h = divmod(core, 2)
        out[b, :, h * QH:(h + 1) * QH] = res.results[core]["out"].T
    out += bv[None, :, None]
    return np.ascontiguousarray(out.reshape(B, CV, HH, WW))
